# revision 1
# baseline (speedup 1.0000x reference)
"""TRN2 Bass kernel for soft 2D polygon rasterization (1024x1024, 64-edge polygon).

Strategy (one SPMD program on 8 cores, per-core behavior fully data-driven):
  - Layout: x (columns) on partitions, y (rows) on the free axis. The image is
    split into 64 tiles of [128 cols x 128 rows]; each core processes 8,
    assigned by a host-side load-balancing local search that minimizes the
    padded per-phase slot maxima (all cores run the same instruction stream).
  - Inside/outside parity: host builds a per-column histogram of edge-crossing
    rows with alternating +1/-1 weights (sorted order), so a prefix sum along y
    gives parity (0/1) directly. The prefix sum is one f32 matmul per tile
    against a triangular 0/1 matrix on the otherwise-idle TensorEngine. The
    bbox+threshold band mask is folded in as +-131072 histogram step entries
    (y) and per-column offsets (x), driving sd2 below the -450 zero cutoff for
    out-of-band pixels (which are provably >= 30 px from the boundary).
  - Distance: sigmoid(+-d2) is within e^-30 ~ 9e-14 of exact 1.0/0.0 once
    d2 >= 30, far below the scale-relative absmax gate, so only pixels within
    ~5.5 px of the boundary need the true distance. The host culls, per tile,
    the edges/vertices within reach. Per edge:
    d2_seg >= max(BIG*overshoot, c^2) with equality wherever it matters;
    c^2 and BIG*|overshoot| are single fused ACT ops (func(scale*y + bias[p]))
    -- every 3rd slot computes c^2 on the vector engine instead to balance the
    engines -- combined with one scalar_tensor_tensor (the first slot writes
    d2 directly) and one tensor_tensor min. Vertices: one ACT Square plus one
    fused add-min scalar_tensor_tensor.
  - Finals: sd2 = (parity-0.5)*d2min via one STT per tile; two tiles share a
    [128,256] buffer so one ACT Sigmoid(2*sd2) serves both (the ACT spline
    saturates to exactly 0.0/1.0 at the extremes, so no explicit far-field
    zero test is needed); DMA out. Host reassembles 64 tiles, transposes.
"""
import os
import numpy as np

W = H = 1024
NCORES = 8
OCT_H = 128          # tile rows
NOCT = 8             # tiles per core
SIGMA = 1.0
THRESHOLD = 30.0
BIG = 1e6
R_KEEP = 4.0         # cull radius: d2>=30 saturates to within e^-30 of 0/1,
                     # far below the scale-relative absmax gate

LAST_RESULTS = None  # BassKernelResults of the most recent run (for test harness)


# ---------------------------------------------------------------------------
# host-side geometry prep
# ---------------------------------------------------------------------------

def _host_prep(polygon):
    poly = np.asarray(polygon, dtype=np.float32)
    E = poly.shape[0]
    a = poly
    b = np.roll(poly, -1, axis=0)
    ab = b - a

    # bbox band (exact f32 replication of the reference)
    x_lo = np.float32(np.floor(poly[:, 0].min()))
    y_lo = np.float32(np.floor(poly[:, 1].min()))
    x_hi = np.float32(np.floor(poly[:, 0].max()) + np.float32(1.0))
    y_hi = np.float32(np.floor(poly[:, 1].max()) + np.float32(1.0))
    thr = np.float32(THRESHOLD)
    xband_lo = x_lo - thr
    xband_hi = x_hi + thr
    yband_lo = y_lo - thr
    yband_hi = y_hi + thr

    # ---- signed crossing histogram (exact f32 semantics) ----
    PX = np.arange(W, dtype=np.float32)[None, :]
    a0 = a[:, 0:1]; a1 = a[:, 1:2]; b0 = b[:, 0:1]
    ab0 = ab[:, 0:1]; ab1 = ab[:, 1:2]
    crosses = (a0 <= PX) != (b0 <= PX)                       # [E, W]
    safe_dx = np.where(ab0 == np.float32(0.0), np.float32(1.0), ab0)
    with np.errstate(over='ignore', invalid='ignore'):
        yint = a1 + (PX - a0) * ab1 / safe_dx                # [E, W] f32
    bins = np.where(crosses, np.ceil(yint.astype(np.float64)), np.inf)
    bins = np.where(bins < 0, 0.0, bins)                     # clamp below
    bins = np.where(bins > H - 1, np.inf, bins)              # >1023 never hits
    srt = np.sort(bins, axis=0)                              # per column asc
    sign = np.where((np.arange(E)[:, None] % 2) == 0, 1.0, -1.0)
    hist = np.zeros((H, W), dtype=np.float32)
    valid = np.isfinite(srt)
    kk = srt[valid].astype(np.int64)
    jj = np.broadcast_to(np.arange(W)[None, :], (E, W))[valid]
    np.add.at(hist, (kk, jj), np.broadcast_to(sign, (E, W))[valid])
    # parity below row r0: number of bins < r0 mod 2 == signed prefix (0/1)
    csum = np.cumsum(hist, axis=0)                           # parity at row i

    # ---- per-(edge, strip) reach culling (f64 geometry) ----
    A = a.astype(np.float64); B = b.astype(np.float64); AB = B - A
    L2 = AB[:, 0] ** 2 + AB[:, 1] ** 2
    Lc = np.sqrt(np.maximum(L2, 1e-12))
    good = L2 > 1e-9

    # per octant (strip s, oct o): lists of edge ids and vertex ids
    NO = H // OCT_H
    oct_edges = [[[] for _ in range(NO)] for _ in range(8)]
    oct_verts = [[[] for _ in range(NO)] for _ in range(8)]
    for s in range(8):
        xr0, xr1 = s * 128, s * 128 + 127
        for e in range(E):
            ax, ay = A[e]; bx, by = B[e]
            if good[e]:
                lo, hi = min(ax, bx), max(ax, bx)
                if not (hi < xr0 - R_KEEP or lo > xr1 + R_KEEP):
                    ts = [0.0, 1.0]
                    if abs(bx - ax) > 1e-12:
                        for xc in (xr0 - R_KEEP, xr1 + R_KEEP):
                            t = (xc - ax) / (bx - ax)
                            if 0.0 < t < 1.0:
                                ts.append(t)
                    ts = [t for t in ts
                          if xr0 - R_KEEP - 1e-9 <= ax + (bx - ax) * t <= xr1 + R_KEEP + 1e-9]
                    if ts:
                        ys = [ay + (by - ay) * t for t in ts]
                        ylo = max(0, int(np.floor(min(ys) - R_KEEP)))
                        yhi = min(H - 1, int(np.ceil(max(ys) + R_KEEP)))
                        if ylo <= yhi:
                            for o in range(NO):
                                if max(ylo, o * OCT_H) <= min(yhi, o * OCT_H + OCT_H - 1):
                                    oct_edges[s][o].append(e)
            if xr0 - R_KEEP <= ax <= xr1 + R_KEEP:
                ylo = max(0, int(np.floor(ay - R_KEEP)))
                yhi = min(H - 1, int(np.ceil(ay + R_KEEP)))
                for o in range(NO):
                    if max(ylo, o * OCT_H) <= min(yhi, o * OCT_H + OCT_H - 1):
                        oct_verts[s][o].append(e)

    # ---- octant -> (core, phase) assignment ----
    # The SPMD program pads each phase to the max (edge, vertex) slot counts over
    # cores, so the objective is sum_k (cE*maxE_k + cV*maxV_k) after sorting
    # each core's octants by cost. LPT start + pairwise-swap local search.
    octs = [(s, o) for s in range(8) for o in range(NO)]
    nE = {so: len(oct_edges[so[0]][so[1]]) for so in octs}
    nV = {so: len(oct_verts[so[0]][so[1]]) for so in octs}
    cE, cV = 2.0, 1.0
    cost = {so: cE * nE[so] + cV * nV[so] for so in octs}

    def padded_cost(assign):
        tot = 0.0
        ranked = [sorted(a, key=lambda so: -cost[so]) for a in assign]
        for k in range(NOCT):
            tot += cE * max(nE[r[k]] for r in ranked)
            tot += cV * max(nV[r[k]] for r in ranked)
        return tot

    order = sorted(octs, key=lambda so: -cost[so])
    core_load = [0.0] * NCORES
    assign = [[] for _ in range(NCORES)]
    for so in order:
        cands = [c for c in range(NCORES) if len(assign[c]) < NOCT]
        c = min(cands, key=lambda c: core_load[c])
        assign[c].append(so)
        core_load[c] += cost[so]
    best = padded_cost(assign)
    rng = np.random.default_rng(0)
    for _ in range(8000):
        c1, c2 = rng.integers(0, NCORES, 2)
        if c1 == c2:
            continue
        i1, i2 = rng.integers(0, NOCT, 2)
        assign[c1][i1], assign[c2][i2] = assign[c2][i2], assign[c1][i1]
        newc = padded_cost(assign)
        if newc <= best:
            best = newc
        else:
            assign[c1][i1], assign[c2][i2] = assign[c2][i2], assign[c1][i1]
    core_octs = [sorted(a, key=lambda so: -cost[so]) for a in assign]

    S = [max(len(oct_edges[core_octs[c][k][0]][core_octs[c][k][1]])
             for c in range(NCORES)) for k in range(NOCT)]
    V = [max(len(oct_verts[core_octs[c][k][0]][core_octs[c][k][1]])
             for c in range(NCORES)) for k in range(NOCT)]

    # ---- per-core input tensors ----
    # coef layout per phase k: [scC, bC, scM, bM, bigl2] * S[k] then [bV, kx]*V[k]
    # then [sc05]
    # Band masking is folded into the parity matmul: out-of-band rows/columns
    # get a -BANDK offset in par (via extra histogram step entries for y, via
    # sc05 for x), which drives sd2 below the -450 zero-test. Out-of-band
    # pixels are >=30 px from the polygon so their computed d2 >= ~104 and
    # BANDK*d2 is always large enough.
    BANDK = 131072.0
    ncol = sum(5 * S[k] + 2 * V[k] + 1 for k in range(NOCT))
    xs_all = np.arange(W, dtype=np.float64)
    # first/last in-band rows (integer pixel coords, f32-exact values)
    r_lo = int(np.ceil(float(yband_lo)))
    r_hi = int(np.floor(float(yband_hi)))
    in_maps = []
    for c in range(NCORES):
        coef = np.zeros((128, ncol), dtype=np.float32)
        histc = np.zeros((NOCT, OCT_H, 128), dtype=np.float32)
        col = 0
        for k in range(NOCT):
            s, o = core_octs[c][k]
            i0 = o * OCT_H
            xs = xs_all[s * 128:(s + 1) * 128]
            elist = oct_edges[s][o]
            vlist = oct_verts[s][o]
            for si in range(S[k]):
                if si < len(elist):
                    e = elist[si]
                    L = Lc[e]
                    scC = -AB[e, 0] / L
                    bC = ((xs - A[e, 0]) * AB[e, 1] + A[e, 1] * AB[e, 0]) / L + scC * i0
                    scM = BIG * AB[e, 1] / L
                    bM = (BIG * (((xs - A[e, 0]) * AB[e, 0] - A[e, 1] * AB[e, 1]) / L
                                 - L / 2.0) + scM * i0)
                    bigl2 = BIG * L / 2.0
                else:  # dummy: candidate = 4000 everywhere (saturated, bounded)
                    scC = 0.0; bC = np.full(128, 60.0); scM = 0.0
                    bM = np.full(128, 4000.0); bigl2 = 0.0
                coef[:, col + 0] = scC
                coef[:, col + 1] = bC
                coef[:, col + 2] = scM
                coef[:, col + 3] = bM
                coef[:, col + 4] = bigl2
                col += 5
            for vi in range(V[k]):
                if vi < len(vlist):
                    e = vlist[vi]
                    coef[:, col + 0] = i0 - A[e, 1]
                    coef[:, col + 1] = np.square(xs - A[e, 0])
                else:
                    coef[:, col + 0] = 200.0   # sqv >= 4e4: never the min
                    coef[:, col + 1] = 0.0
                col += 2
            base = np.mod(csum[i0 - 1, s * 128:(s + 1) * 128], 2.0) if i0 > 0 \
                else np.zeros(128)
            # y-band step entries (local rows), plus constant part
            hloc = np.ascontiguousarray(hist[i0:i0 + OCT_H, s * 128:(s + 1) * 128])
            base_const = -BANDK
            rl, rh1 = r_lo - i0, r_hi - i0 + 1
            if rl <= 0:
                base_const += BANDK
            elif rl <= OCT_H - 1:
                hloc[rl, :] += BANDK
            if rh1 <= 0:
                base_const -= BANDK
            elif rh1 <= OCT_H - 1:
                hloc[rh1, :] -= BANDK
            xsf = xs.astype(np.float32)
            xg = np.where((xsf >= xband_lo) & (xsf <= xband_hi), 0.0, -BANDK)
            coef[:, col + 0] = 0.5 - base - base_const - xg
            col += 1
            histc[k] = hloc
        in_maps.append({
            "coef": coef,
            "hist": histc.reshape(NOCT * OCT_H, 128),
        })
    return in_maps, core_octs, S, V, ncol


# ---------------------------------------------------------------------------
# device program
# ---------------------------------------------------------------------------

def _build_program(S, V, ncol):
    import concourse.bacc as bacc
    import concourse.mybir as mybir
    from concourse.tile import TileContext

    F32 = mybir.dt.float32
    I32 = mybir.dt.int32
    BF16 = mybir.dt.bfloat16
    AF = mybir.ActivationFunctionType
    OP = mybir.AluOpType

    nc = bacc.Bacc()
    coef_in = nc.declare_dram_parameter("coef", [128, ncol], F32, isOutput=False)
    hist_in = nc.declare_dram_parameter("hist", [NOCT * OCT_H, 128], F32, isOutput=False)
    out_dram = nc.declare_dram_parameter("out", [NOCT, 128, OCT_H], F32, isOutput=True)

    with TileContext(nc) as tc:
        with tc.tile_pool(name="const", bufs=1) as cpool, \
             tc.tile_pool(name="work", bufs=8) as wpool, \
             tc.tile_pool(name="acc", bufs=4) as apool, \
             tc.tile_pool(name="ps", bufs=4, space="PSUM") as psum:

            # per-phase coef slices so phase 0 can start as soon as possible
            coef = cpool.tile([128, ncol], F32)
            cc = 0
            for k in range(NOCT):
                w = 5 * S[k] + 2 * V[k] + 1
                nc.sync.dma_start(out=coef[:, cc:cc + w], in_=coef_in[:, cc:cc + w])
                cc += w

            # warmup: trigger the ACT table load (sigmoid_and_others covers
            # Square/Abs/Sigmoid) while input DMAs are in flight
            warm = cpool.tile([128, 1], F32)
            nc.vector.memset(warm[:], 0.0)
            nc.scalar.activation(warm[:], warm[:], AF.Sigmoid, bias=0.0, scale=1.0)

            # Yr = iota f32 (row index within octant)
            yi = cpool.tile([128, OCT_H], I32)
            nc.gpsimd.iota(yi[:], pattern=[[1, OCT_H]], base=0, channel_multiplier=0)
            yr = cpool.tile([128, OCT_H], F32)
            nc.vector.tensor_copy(out=yr[:], in_=yi[:])

            # U triangular [128, 128] f32: U[kk, ii] = (kk <= ii)
            ui = cpool.tile([128, OCT_H], I32)
            nc.gpsimd.iota(ui[:], pattern=[[1, OCT_H]], base=0,
                           channel_multiplier=-1)
            ubf = cpool.tile([128, OCT_H], F32)
            nc.vector.tensor_scalar(out=ubf[:], in0=ui[:], scalar1=0, scalar2=None,
                                    op0=OP.is_ge)

            col = 0
            for k in range(NOCT):
                # parity prefix-sum matmuls
                hk0 = wpool.tile([128, 128], F32, tag="hist0")
                nc.sync.dma_start(out=hk0[:],
                                  in_=hist_in[k * OCT_H:(k + 1) * OCT_H, :])
                par = psum.tile([128, OCT_H], F32, tag="par")
                nc.tensor.matmul(par[:], lhsT=hk0[:], rhs=ubf[:],
                                 start=True, stop=True)

                d2 = apool.tile([128, OCT_H], F32, tag="d2")
                if S[k] == 0:
                    nc.vector.memset(d2[:], 1000.0)

                for si in range(S[k]):
                    m = wpool.tile([128, OCT_H], F32, tag="m")
                    nc.scalar.activation(m[:], yr[:], AF.Abs,
                                         bias=coef[:, col + 3:col + 4],
                                         scale=coef[:, col + 2:col + 3])
                    c2 = wpool.tile([128, OCT_H], F32, tag="c2")
                    if si % 3 == 2:
                        # DVE path for (scC*y + bC)^2 (TS 2x-mode + TT self-mult)
                        u = wpool.tile([128, OCT_H], F32, tag="u")
                        nc.vector.tensor_scalar(
                            out=u[:], in0=yr[:], scalar1=coef[:, col + 0:col + 1],
                            scalar2=coef[:, col + 1:col + 2], op0=OP.mult, op1=OP.add)
                        nc.vector.tensor_tensor(out=c2[:], in0=u[:], in1=u[:],
                                                op=OP.mult)
                    else:
                        nc.scalar.activation(c2[:], yr[:], AF.Square,
                                             bias=coef[:, col + 1:col + 2],
                                             scale=coef[:, col + 0:col + 1])
                    if si == 0:
                        # first candidate initializes d2 directly
                        nc.vector.scalar_tensor_tensor(
                            out=d2[:], in0=m[:], scalar=coef[:, col + 4:col + 5],
                            in1=c2[:], op0=OP.subtract, op1=OP.max)
                    else:
                        cand = wpool.tile([128, OCT_H], F32, tag="cand")
                        nc.vector.scalar_tensor_tensor(
                            out=cand[:], in0=m[:], scalar=coef[:, col + 4:col + 5],
                            in1=c2[:], op0=OP.subtract, op1=OP.max)
                        nc.vector.tensor_tensor(out=d2[:], in0=d2[:], in1=cand[:],
                                                op=OP.min)
                    col += 5

                for vi in range(V[k]):
                    sqv = wpool.tile([128, OCT_H], F32, tag="sqv")
                    nc.scalar.activation(sqv[:], yr[:], AF.Square,
                                         bias=coef[:, col + 0:col + 1], scale=1.0)
                    nc.vector.scalar_tensor_tensor(
                        out=d2[:], in0=sqv[:], scalar=coef[:, col + 1:col + 2],
                        in1=d2[:], op0=OP.add, op1=OP.min)
                    col += 2

                # finals: sd2 halves of a phase pair share one tile, one sigmoid
                if k % 2 == 0:
                    sd2p = apool.tile([128, 2 * OCT_H], F32, tag="sd2p")
                nc.vector.scalar_tensor_tensor(
                    out=sd2p[:, (k % 2) * OCT_H:(k % 2 + 1) * OCT_H],
                    in0=par[:], scalar=coef[:, col + 0:col + 1],
                    in1=d2[:], op0=OP.subtract, op1=OP.mult)
                if k % 2 == 1:
                    val = wpool.tile([128, 2 * OCT_H], F32, tag="val")
                    nc.scalar.activation(val[:], sd2p[:], AF.Sigmoid,
                                         bias=0.0, scale=2.0)
                    nc.sync.dma_start(out=out_dram[k - 1], in_=val[:, 0:OCT_H])
                    nc.sync.dma_start(out=out_dram[k], in_=val[:, OCT_H:])
                col += 1

    nc.finalize()
    return nc


# ---------------------------------------------------------------------------
# entry point
# ---------------------------------------------------------------------------

def kernel(polygon):
    global LAST_RESULTS
    from concourse.bass_utils import run_bass_kernel_spmd

    in_maps, core_octs, S, V, ncol = _host_prep(polygon)
    nc = _build_program(S, V, ncol)
    trace = bool(int(os.environ.get("KERNEL_TRACE", "0")))
    res = run_bass_kernel_spmd(nc, in_maps, list(range(NCORES)), trace=trace)
    LAST_RESULTS = res

    full = np.zeros((W, H), dtype=np.float32)   # x-major
    for c in range(NCORES):
        o = res.results[c]["out"]
        for k in range(NOCT):
            s, oq = core_octs[c][k]
            full[s * 128:(s + 1) * 128, oq * OCT_H:(oq + 1) * OCT_H] = o[k]
    return np.ascontiguousarray(full.T)

